# revision 17
# baseline (speedup 1.0000x reference)
"""Trainium2 Bass kernel for nn_Branch_2_36386962932308.

Network (per batch, feature-major planes [channels, L=h*w=4096]):
  stage1: Mamba(d=128, di=128, n=2, r=8, conv4) -> LN
  linear: 128->256 + SiLU   (stage-1 LN affine folded into the linear weight)
  stage2: Mamba(d=256, di=256, n=2, r=16, conv4) -> LN (affine applied on device)

Sharding: data-parallel over batch, one batch element per NeuronCore (8 cores).

v3 — chunk-interleaved software pipeline:
  - stage1(chunk k) and stage2(chunk k-2) are processed in the same pipeline
    iteration, so stage2's Tensor/ACT work fills stage1's latency gaps.
  - ACT table phases per iteration: A = [SiLUs], B = [dt chains] + the
    PREVIOUS chunk's LayerNorm input ops (yp->SBUF Identity + Square, which
    are table-neutral).  Nothing late ever heads the ACT queue, so the
    in-order engine queues never head-of-line block.
  - LayerNorm rstd = 1/sqrt(var+eps) computed WITHOUT ACT Ln/Exp: bit-trick
    seed (DVE int ALU) + two Newton steps on the (otherwise idle) GpSimd.
  - out_proj groups are packed as views into [SUB,512] PSUM bank tiles
    (s2: 2x256 halves, s1: 4x128 quarters); yp leaves PSUM via one ACT
    Identity (accum gives sum -> mean), so PSUM never backs up the PE.
  - normalize = single DVE tensor_scalar in 4x mode (bf16 SBUF operands).
  - causal depthwise conv folded into in_proj (4 shifted matmuls); B/C rows
    replicated via DMA row-flatten + Pool partition_broadcast; dA1 = dA0^2
    on Pool; bf16 on all elementwise-heavy paths.
  - LN outputs return to feature-major via PE transpose (the XBAR DMA
    transpose corrupts even lanes on this platform).

Self-contained: hardcodes all shapes; needs only concourse + numpy at runtime.
"""

import os
from contextlib import ExitStack

import numpy as np

import concourse.bass as bass
import concourse.bacc as bacc
import concourse.mybir as mybir
import concourse.tile as tile
from concourse.bass_utils import run_bass_kernel_spmd

F32 = mybir.dt.float32
BF16 = mybir.dt.bfloat16
I32 = mybir.dt.int32
AF = mybir.ActivationFunctionType
ALU = mybir.AluOpType

NCORES = 8
LN_EPS = 1e-5
CH = 512          # pipeline column chunk (one PSUM bank at fp32)
SUB = 128         # out_proj / LN subchunk (time-major tile height)
MAGIC = 0x5F3759DF

last_exec_time_ns = None
last_results = None


def _patch_act_tables():
    """Make natural_log_exp_and_others the only table set containing Exp and
    Ln, so bacc's table-load placement keeps one set resident through each
    B phase instead of swapping between exp_and_others and natural_log on
    every Exp<->Ln transition (~2.7us per swap)."""
    import functools
    import concourse.hw_specs as hw_specs
    if getattr(hw_specs.get_activation_tables, "_lnexp_patched", False):
        return
    orig = hw_specs.get_activation_tables

    @functools.cache
    def patched(arch):
        tables = {k: set(v) for k, v in orig(arch).items()}
        for name, fns in tables.items():
            if name != 'natural_log_exp_and_others':
                fns.discard(AF.Exp)
                fns.discard(AF.Ln)
        return tables

    patched._lnexp_patched = True
    hw_specs.get_activation_tables = patched
    bacc.get_activation_tables = patched


_patch_act_tables()


# ----------------------------------------------------------------------------
# host-side weight preparation
# ----------------------------------------------------------------------------

def _prep_stage(p, d, di, r):
    win = np.asarray(p['win'], np.float32)
    b_in = np.asarray(p['bin'], np.float32)
    cw = np.asarray(p['cw'], np.float32)        # [di, 1, 4]
    cb = np.asarray(p['cb'], np.float32)
    wx = np.asarray(p['wx'], np.float32)        # [r+4, di]
    wdt = np.asarray(p['wdt'], np.float32)      # [di, r]
    bdt = np.asarray(p['bdt'], np.float32)
    alog = np.asarray(p['alog'], np.float32)    # [di, 2]
    dd = np.asarray(p['dd'], np.float32)
    wout = np.asarray(p['wout'], np.float32)    # [dout, di]

    winx, winz = win[:di], win[di:]
    w_k = np.stack([np.ascontiguousarray((cw[:, 0, k:k + 1] * winx).T)
                    for k in range(4)])          # [4, d, di]
    wz = np.ascontiguousarray(winz.T)            # [d, di]
    wxT = np.ascontiguousarray(wx.T)             # [di, r+4]
    wdtT = np.ascontiguousarray(wdt.T)           # [r, di]
    woutT = np.ascontiguousarray(wout.T)         # [di, dout]

    S = cw[:, 0, :].sum(1)
    silu_bias = cb + S * b_in[:di]
    bz = b_in[di:]
    A = -np.exp(alog)                            # [di, 2] (negative)
    corr = np.stack([-(cw[:, 0, :3 - t].sum(1)) * b_in[:di] for t in range(3)], 1)
    cols = [silu_bias, bz, bdt, A[:, 0], A[:, 1], dd,
            corr[:, 0], corr[:, 1], corr[:, 2]]
    return w_k, wz, wxT, wdtT, woutT, np.stack(cols, 1).astype(np.float32)


def prep_weights(inputs):
    s1 = {k[3:]: inputs[k] for k in inputs if k.startswith('s1_')}
    s2 = {k[3:]: inputs[k] for k in inputs if k.startswith('s2_')}
    w1k, w1z, wx1, wdt1, wout1, cols1 = _prep_stage(s1, 128, 128, 8)
    w2k, w2z, wx2, wdt2, wout2, cols2 = _prep_stage(s2, 256, 256, 16)
    lnw2 = np.asarray(s2['lnw'], np.float32)
    lnb2 = np.asarray(s2['lnb'], np.float32)
    cols2 = np.concatenate([cols2, lnw2[:, None], lnb2[:, None]], 1)
    cols2 = np.ascontiguousarray(cols2, dtype=np.float32)

    bfdt = mybir.dt.np(BF16)
    lin_w = np.asarray(inputs['lin_w'], np.float32)
    lin_b = np.asarray(inputs['lin_b'], np.float32)
    lnw1 = np.asarray(s1['lnw'], np.float32)
    lnb1 = np.asarray(s1['lnb'], np.float32)
    linw = np.ascontiguousarray((lin_w * lnw1[None, :]).T)
    linb = (lin_w @ lnb1 + lin_b).astype(np.float32)[:, None]

    return {
        'idenb': np.eye(128, dtype=np.float32).astype(bfdt),
        'w1k': w1k, 'w1z': w1z, 'wx1': wx1.astype(bfdt),
        'wdt1': wdt1.astype(bfdt),
        'wout1': wout1.astype(bfdt), 'cols1': cols1,
        'w2k': w2k.astype(bfdt), 'w2z': w2z.astype(bfdt),
        'wx2': wx2.astype(bfdt), 'wdt2': wdt2.astype(bfdt),
        'wout2': wout2.astype(bfdt), 'cols2': cols2,
        'linw': linw.astype(bfdt), 'linb': linb,
    }


# ----------------------------------------------------------------------------
# device program
# ----------------------------------------------------------------------------

F32R = mybir.dt.float32r


def _tile(pool, shape, dtype, tag, bufs=None):
    return pool.tile(shape, dtype, tag=tag, name=tag, bufs=bufs)


def _mmr(nc, out, lhsT, rhs, **kw):
    """fp32 matmul via float32r bitcast: single-pass on the PE."""
    nc.tensor.matmul(out, lhsT.bitcast(F32R), rhs.bitcast(F32R), **kw)


def _mmb(nc, out, lhsT, rhs, **kw):
    """all-bf16 matmul (full PE rate at any N)."""
    nc.tensor.matmul(out, lhsT, rhs, **kw)


class _ActChain:
    """Groups ACT instructions into table-set phases separated by no-op
    barrier instructions, so the scheduler can reorder freely within a phase
    (same table set) but cannot interleave phases (which would make bacc
    insert a ~2.7us ACT table load per out-of-phase function switch)."""

    def __init__(self, nc, bar_tile):
        self.nc = nc
        self.bar_tile = bar_tile
        self.group = []
        self.barrier = None

    def new_group(self):
        from concourse.tile_rust import add_dep_helper
        if not self.group:
            return
        bar = self.nc.scalar.activation(self.bar_tile[:], self.bar_tile[:],
                                        AF.Identity)
        barc = bar.ins if hasattr(bar, 'ins') else bar
        for op in self.group:
            add_dep_helper(barc, op, sync=False, reason="act phase barrier")
        self.barrier = barc
        self.group = []

    def __call__(self, *args, **kwargs):
        from concourse.tile_rust import add_dep_helper
        inst = self.nc.scalar.activation(*args, **kwargs)
        cur = inst.ins if hasattr(inst, 'ins') else inst
        if self.barrier is not None:
            add_dep_helper(cur, self.barrier, sync=False,
                           reason="act phase order")
        self.group.append(cur)
        return inst


class _Stage:
    """One Mamba stage's per-chunk pipeline pieces."""

    def __init__(self, nc, act, pools, P_in, P, r, dout, in_planes,
                 wk, wz, wx, wdt, wout, cols, mm_in, ps_y, emit, flush,
                 pfx=""):
        self.nc = nc
        self.act = act
        self.pools = pools
        self.pfx = pfx
        self.P_in, self.P, self.r, self.dout = P_in, P, r, dout
        self.in_planes = in_planes
        self.wk, self.wz, self.wx, self.wdt, self.wout, self.cols = \
            wk, wz, wx, wdt, wout, cols
        self.mm_in = mm_in
        self.ps_y = ps_y
        self.emit = emit
        self.flush = flush
        self.hs_prev = [[None] * P, [None] * P]
        self.st = {}                  # chunk k -> dict of live tiles

    # ---- A phase: in_proj (conv folded) + z + SiLU ----------------------
    def front_a(self, k):
        nc, act = self.nc, self.act
        ps_mm = self.pools['mm']
        c0 = k * CH
        s = {}
        xc = [None] * self.P
        sz = [None] * self.P
        for mi in range(self.P):
            ms = slice(mi * 128, (mi + 1) * 128)
            xc_ps = _tile(ps_mm, [128, CH], F32, "mm", 3)
            nmm = 4 * self.P_in
            i = 0
            for kk in range(4):
                for kt in range(self.P_in):
                    self.mm_in(nc, xc_ps[:], self.wk[kk][kt][:, ms],
                               self.in_planes[kt][:, c0 + kk: c0 + kk + CH],
                               start=(i == 0), stop=(i == nmm - 1))
                    i += 1
            if c0 == 0:
                nc.vector.tensor_add(xc_ps[:, 0:3], xc_ps[:, 0:3],
                                     self.cols[mi][:, 6:9])
            t_xc = _tile(self.pools['sb'], [128, CH], BF16,
                         self.pfx + "xc", 2 * self.P + 2)
            act(t_xc[:], xc_ps[:], AF.Silu, bias=self.cols[mi][:, 0:1])
            xc[mi] = t_xc

            z_ps = _tile(ps_mm, [128, CH], F32, "mm", 3)
            for kt in range(self.P_in):
                self.mm_in(nc, z_ps[:], self.wz[kt][:, ms],
                           self.in_planes[kt][:, c0 + 3: c0 + 3 + CH],
                           start=(kt == 0), stop=(kt == self.P_in - 1))
            t_sz = _tile(self.pools['sb'], [128, CH], BF16,
                         self.pfx + "sz", 2 * self.P + 2)
            act(t_sz[:], z_ps[:], AF.Silu, bias=self.cols[mi][:, 1:2])
            sz[mi] = t_sz
        s['xc'], s['sz'] = xc, sz
        self.st[k] = s

    # ---- B phase: wx/dt projections + dt chain --------------------------
    def front_b(self, k):
        nc, act = self.nc, self.act
        sb, ps_mm = self.pools['sb'], self.pools['mm']
        rw = self.r + 4
        s = self.st[k]
        xdbl_ps = _tile(ps_mm, [128, CH], F32, "mm", 3)
        for kt in range(self.P_in):
            _mmb(nc, xdbl_ps[:rw, :], self.wx[kt][:], s['xc'][kt][:],
                 start=(kt == 0), stop=(kt == self.P_in - 1))
        xdbl = _tile(sb, [rw, CH], BF16, self.pfx + "xdbl", 2)
        act(xdbl[:], xdbl_ps[:rw, :], AF.Identity)
        s['xdbl'] = xdbl

        dt_sb = []
        dA_sb = [[None] * self.P, [None] * self.P]
        for mi in range(self.P):
            ms = slice(mi * 128, (mi + 1) * 128)
            dt_ps = _tile(ps_mm, [128, CH], F32, "mm", 3)
            _mmb(nc, dt_ps[:], self.wdt[:, ms], xdbl[:self.r, :])
            t_e = _tile(sb, [128, CH], F32, "scr1", 3)
            act(t_e[:], dt_ps[:], AF.Exp, bias=self.cols[mi][:, 2:3])
            t_dt = _tile(sb, [128, CH], BF16, self.pfx + "dt", self.P + 2)
            act(t_dt[:], t_e[:], AF.Ln, bias=1.0)
            dt_sb.append(t_dt)
            t_dA0 = _tile(sb, [128, CH], F32, self.pfx + "dA0", self.P + 1)
            act(t_dA0[:], t_dt[:], AF.Exp, scale=self.cols[mi][:, 3:4])
            dA_sb[0][mi] = t_dA0
            t_dA1 = _tile(sb, [128, CH], F32, self.pfx + "dA1", self.P + 1)
            nc.gpsimd.tensor_mul(t_dA1[:], t_dA0[:], t_dA0[:])
            dA_sb[1][mi] = t_dA1
        s['dt'], s['dA'] = dt_sb, dA_sb

    # ---- DVE: gate/scan chain -------------------------------------------
    def dve_chain(self, k):
        nc = self.nc
        sb = self.pools['sb']
        c0 = k * CH
        s = self.st[k]
        rw = self.r
        rows = _tile(sb, [1, 4 * CH], BF16, "rows", 2)
        nc.scalar.dma_start(rows[:], s['xdbl'][rw:rw + 4, :])
        rep_all = _tile(sb, [128, 4 * CH], BF16, "repall", 3)
        nc.gpsimd.partition_broadcast(rep_all[:], rows[:])
        rep = [rep_all[:, j * CH:(j + 1) * CH] for j in range(4)]
        hs = [[None] * self.P, [None] * self.P]
        for mi in range(self.P):
            t_u = _tile(sb, [128, CH], BF16, self.pfx + "scr2", 3)
            nc.vector.tensor_mul(t_u[:], s['dt'][mi][:], s['xc'][mi][:])
            for n in range(2):
                t_dbu = _tile(sb, [128, CH], BF16, self.pfx + f"dbu{n}", 2)
                nc.vector.tensor_mul(t_dbu[:], t_u[:], rep[n])
                t_hs = _tile(sb, [128, CH], BF16, self.pfx + f"hs{n}",
                             2 * self.P + 1)
                init = (0.0 if c0 == 0 else
                        self.hs_prev[n][mi][:, CH - 1:CH])
                nc.vector.tensor_tensor_scan(
                    t_hs[:], s['dA'][n][mi][:], t_dbu[:], init,
                    ALU.mult, ALU.add)
                hs[n][mi] = t_hs
                self.hs_prev[n][mi] = t_hs

        yg = []
        for mi in range(self.P):
            t_m0 = _tile(sb, [128, CH], BF16, self.pfx + "m0", 2)
            nc.vector.tensor_mul(t_m0[:], hs[0][mi][:], rep[2])
            t_y = _tile(sb, [128, CH], BF16, self.pfx + "y", 2)
            nc.vector.tensor_mul(t_y[:], hs[1][mi][:], rep[3])
            nc.vector.tensor_add(t_y[:], t_y[:], t_m0[:])
            t_dx = _tile(sb, [128, CH], BF16, self.pfx + "dx", 2)
            nc.vector.tensor_scalar(t_dx[:], s['xc'][mi][:],
                                    self.cols[mi][:, 5:6], None,
                                    ALU.mult, ALU.bypass)
            nc.vector.tensor_add(t_y[:], t_y[:], t_dx[:])
            t_yg = _tile(sb, [128, CH], BF16, self.pfx + "yg",
                         2 * self.P + 3)
            nc.vector.tensor_mul(t_yg[:], t_y[:], s['sz'][mi][:])
            yg.append(t_yg)
        s['yg'] = yg

    # ---- PE: out_proj into packed PSUM bank tiles -----------------------
    def out_proj(self, k):
        nc = self.nc
        s = self.st[k]
        views = []
        n_banks = (4 * self.dout) // 512       # s1: 1, s2: 2
        per_bank = 512 // self.dout            # s1: 4, s2: 2
        for b in range(n_banks):
            bank = _tile(self.ps_y, [SUB, 512], F32, "yb", 2)
            for h in range(per_bank):
                g = b * per_bank + h
                cs = slice(g * SUB, (g + 1) * SUB)
                dst = bank[:, h * self.dout:(h + 1) * self.dout]
                for mi in range(self.P):
                    nc.tensor.matmul(dst, s['yg'][mi][:, cs],
                                     self.wout[mi][:],
                                     start=(mi == 0), stop=(mi == self.P - 1))
                views.append(dst)
        s['yp'] = views

    # ---- B phase (retarded): yp -> SBUF + LN stats ----------------------
    def ln_acts(self, k):
        sb = self.pools['sb']
        s = self.st[k]
        mu4 = _tile(sb, [SUB, 4], F32, self.pfx + "mu4", 2)
        ssq4 = _tile(sb, [SUB, 4], F32, self.pfx + "ssq4", 2)
        ypsb = []
        for g in range(4):
            t_y = _tile(sb, [SUB, self.dout], BF16, self.pfx + "ypsb", 8)
            self.act(t_y[:], s['yp'][g], AF.Identity,
                     accum_out=mu4[:, g:g + 1])
            scr = _tile(sb, [SUB, self.dout], BF16, self.pfx + "scr", 2)
            self.act(scr[:], t_y[:], AF.Square, accum_out=ssq4[:, g:g + 1])
            ypsb.append(t_y)
        s['ypsb'], s['mu4'], s['ssq4'] = ypsb, mu4, ssq4

    # ---- Pool/DVE: rstd chain + normalize + emit ------------------------
    def ln_tail(self, k):
        nc = self.nc
        sb = self.pools['sb']
        s = self.st.pop(k)
        c0 = k * CH
        dout = self.dout
        mu = _tile(sb, [SUB, 4], F32, self.pfx + "mu", 2)
        nc.gpsimd.tensor_scalar(mu[:], s['mu4'][:], 1.0 / dout, None,
                                ALU.mult, ALU.bypass)
        musq = _tile(sb, [SUB, 4], F32, self.pfx + "musq", 2)
        nc.gpsimd.tensor_mul(musq[:], mu[:], mu[:])
        veps = _tile(sb, [SUB, 4], F32, self.pfx + "veps", 2)
        nc.gpsimd.tensor_scalar(veps[:], s['ssq4'][:], 1.0 / dout, LN_EPS,
                                ALU.mult, ALU.add)
        nc.gpsimd.tensor_sub(veps[:], veps[:], musq[:])
        i1 = _tile(sb, [SUB, 4], I32, self.pfx + "i1", 2)
        nc.vector.tensor_scalar(i1[:], veps[:].bitcast(I32), 1, None,
                                ALU.arith_shift_right, ALU.bypass)
        nc.vector.tensor_scalar(i1[:], i1[:], 0xFFFFFFFF, None,
                                ALU.bitwise_xor, ALU.bypass)
        nc.vector.tensor_scalar(i1[:], i1[:], MAGIC + 1, None,
                                ALU.add, ALU.bypass)
        r0 = i1[:].bitcast(F32)
        vh = _tile(sb, [SUB, 4], F32, self.pfx + "vh", 2)
        nc.gpsimd.tensor_scalar(vh[:], veps[:], -0.5, None,
                                ALU.mult, ALU.bypass)
        for it in range(2):
            t_nr = _tile(sb, [SUB, 4], F32, self.pfx + f"nr{it}", 2)
            nc.gpsimd.tensor_mul(t_nr[:], r0, vh[:])
            nc.gpsimd.tensor_mul(t_nr[:], t_nr[:], r0)
            nc.gpsimd.tensor_scalar(t_nr[:], t_nr[:], 1.5, None,
                                    ALU.add, ALU.bypass)
            r1 = _tile(sb, [SUB, 4], F32, self.pfx + f"rst{it}", 2)
            nc.gpsimd.tensor_mul(r1[:], r0, t_nr[:])
            r0 = r1[:]
        for g in range(4):
            tn = _tile(sb, [SUB, dout], BF16, self.pfx + "tn", 2)
            nc.vector.tensor_scalar(tn[:], s['ypsb'][g][:],
                                    mu[:, g:g + 1], r0[:, g:g + 1],
                                    ALU.subtract, ALU.mult)
            self.emit(tn, c0, g)
        self.flush(c0)


def build_program(L=4096):
    nc = bacc.Bacc()
    dp = nc.declare_dram_parameter
    x_d = dp("x", [128, L], F32R, isOutput=False)
    w1k_d = dp("w1k", [4, 128, 128], F32R, isOutput=False)
    w1z_d = dp("w1z", [128, 128], F32R, isOutput=False)
    wx1_d = dp("wx1", [128, 12], BF16, isOutput=False)
    wdt1_d = dp("wdt1", [8, 128], BF16, isOutput=False)
    wout1_d = dp("wout1", [128, 128], BF16, isOutput=False)
    cols1_d = dp("cols1", [128, 9], F32, isOutput=False)
    w2k_d = dp("w2k", [4, 256, 256], BF16, isOutput=False)
    w2z_d = dp("w2z", [256, 256], BF16, isOutput=False)
    wx2_d = dp("wx2", [256, 20], BF16, isOutput=False)
    wdt2_d = dp("wdt2", [16, 256], BF16, isOutput=False)
    wout2_d = dp("wout2", [256, 256], BF16, isOutput=False)
    cols2_d = dp("cols2", [256, 11], F32, isOutput=False)
    linw_d = dp("linw", [128, 256], BF16, isOutput=False)
    linb_d = dp("linb", [256, 1], F32, isOutput=False)
    iden_d = dp("idenb", [128, 128], BF16, isOutput=False)
    out_d = dp("out", [256, L], BF16, isOutput=True)

    dma = nc.sync.dma_start
    NCH = L // CH

    with tile.TileContext(nc) as tc, ExitStack() as ctx:
        consts = ctx.enter_context(tc.tile_pool(name="consts", bufs=1))
        planes = ctx.enter_context(tc.tile_pool(name="planes", bufs=1))
        sb = ctx.enter_context(tc.tile_pool(name="sb", bufs=2))
        ps_mm = ctx.enter_context(
            tc.tile_pool(name="psmm", bufs=3, space=bass.MemorySpace.PSUM))
        ps_y1 = ctx.enter_context(
            tc.tile_pool(name="psy1", bufs=2, space=bass.MemorySpace.PSUM))
        ps_y2 = ctx.enter_context(
            tc.tile_pool(name="psy2", bufs=2, space=bass.MemorySpace.PSUM))
        ps_tf = ctx.enter_context(
            tc.tile_pool(name="pstf", bufs=1, space=bass.MemorySpace.PSUM))
        pools = {'sb': sb, 'mm': ps_mm}

        xpad = planes.tile([128, L + 3], F32R, tag="xpad", name="xpad")
        nc.gpsimd.memset(xpad[:, 0:3].bitcast(F32), 0.0)
        for s0 in range(0, L, 2048):
            dma(xpad[:, 3 + s0: 3 + s0 + 2048], x_d[:, s0:s0 + 2048])

        _ld = [0]

        def load(dram_ap, shape, dtype=F32R):
            _ld[0] += 1
            t = consts.tile(shape, dtype, tag=f"w{_ld[0]}", name=f"w{_ld[0]}")
            dma(t[:], dram_ap)
            return t

        w1k_sb = [[load(w1k_d[k], [128, 128])] for k in range(4)]
        w1z_sb = [load(w1z_d[:], [128, 128])]
        wx1_sb = [load(wx1_d[:], [128, 12], BF16)]
        wdt1_sb = load(wdt1_d[:], [8, 128], BF16)
        wout1_sb = [load(wout1_d[:], [128, 128], BF16)]
        cols1_sb = [load(cols1_d[:], [128, 9], F32)]
        w2k_sb = [[load(w2k_d[k, kt * 128:(kt + 1) * 128], [128, 256], BF16)
                   for kt in range(2)] for k in range(4)]
        w2z_sb = [load(w2z_d[kt * 128:(kt + 1) * 128], [128, 256], BF16)
                  for kt in range(2)]
        wx2_sb = [load(wx2_d[kt * 128:(kt + 1) * 128], [128, 20], BF16)
                  for kt in range(2)]
        wdt2_sb = load(wdt2_d[:], [16, 256], BF16)
        wout2_sb = [load(wout2_d[kt * 128:(kt + 1) * 128], [128, 256], BF16)
                    for kt in range(2)]
        cols2_sb = [load(cols2_d[kt * 128:(kt + 1) * 128], [128, 11], F32)
                    for kt in range(2)]
        linw_sb = load(linw_d[:], [128, 256], BF16)
        linb_sb = [load(linb_d[kt * 128:(kt + 1) * 128], [128, 1], F32)
                   for kt in range(2)]

        bar_tile = consts.tile([1, 1], F32, tag="actbar", name="actbar")
        nc.gpsimd.memset(bar_tile[:], 0.0)
        act_chain = _ActChain(nc, bar_tile)
        ident = consts.tile([128, 128], BF16, tag="ident", name="ident")
        dma(ident[:], iden_d[:])

        t1n = planes.tile([128, L], BF16, tag="t1n", name="t1n")
        t2pad = [planes.tile([128, L + 3], BF16, tag=f"t2pad{mi}",
                             name=f"t2pad{mi}") for mi in range(2)]
        for mi in range(2):
            nc.gpsimd.memset(t2pad[mi][:, 0:3], 0.0)

        # ---- emits ----
        def emit1(tn, c0, g):
            tf = _tile(ps_tf, [128, 2 * SUB], BF16, "tf", 1)
            nc.tensor.transpose(tf[:, :SUB], tn[:], ident[:])
            nc.vector.tensor_scalar(
                t1n[:, c0 + g * SUB: c0 + (g + 1) * SUB], tf[:, :SUB],
                1.0, None, ALU.mult, ALU.bypass)

        of_buf = [None, None]

        def emit2(tn, c0, g):
            tf = _tile(ps_tf, [128, 2 * SUB], BF16, "tf", 1)
            for ct in range(2):
                if g == 0:
                    of_buf[ct] = _tile(sb, [128, CH], BF16, f"of{ct}", 2)
                nc.tensor.transpose(tf[:, ct * SUB:(ct + 1) * SUB],
                                    tn[:, ct * 128:(ct + 1) * 128],
                                    ident[:])
                nc.vector.tensor_scalar(
                    of_buf[ct][:, g * SUB:(g + 1) * SUB],
                    tf[:, ct * SUB:(ct + 1) * SUB],
                    cols2_sb[ct][:, 9:10], cols2_sb[ct][:, 10:11],
                    ALU.mult, ALU.add)

        def flush2(c0):
            for ct in range(2):
                dma(out_d[ct * 128:(ct + 1) * 128, c0:c0 + CH],
                    of_buf[ct][:])

        s1 = _Stage(nc, act_chain, pools, 1, 1, 8, 128, [xpad],
                    w1k_sb, w1z_sb, wx1_sb, wdt1_sb, wout1_sb, cols1_sb,
                    _mmr, ps_y1, emit1, lambda c0: None, pfx="a")
        s2 = _Stage(nc, act_chain, pools, 2, 2, 16, 256, t2pad,
                    w2k_sb, w2z_sb, wx2_sb, wdt2_sb, wout2_sb, cols2_sb,
                    _mmb, ps_y2, emit2, flush2, pfx="b")

        def linear_a(k):
            c0 = k * CH
            for mi in range(2):
                ms = slice(mi * 128, (mi + 1) * 128)
                lp = _tile(ps_mm, [128, CH], F32, "mm", 3)
                _mmb(nc, lp[:], linw_sb[:, ms], t1n[:, c0:c0 + CH])
                act_chain(t2pad[mi][:, 3 + c0: 3 + c0 + CH], lp[:],
                          AF.Silu, bias=linb_sb[mi][:, 0:1])

        # ---- the interleaved chunk pipeline ----
        # iter k: s1 front(k); s2 front(k-3); LN stats retarded by TWO
        # chunks (s1: k-2, s2: k-5) so every ACT-chained op is at least one
        # full iteration old when the in-order ACT queue reaches it.
        # out_proj runs at the PE stream front for the same reason.
        for k in range(NCH + 6):
            act_chain.new_group()                 # A phase (Silu table)
            if 0 <= k - 3 < NCH:
                linear_a(k - 3)
            if k < NCH:
                s1.front_a(k)
            if 0 <= k - 3 < NCH:
                s2.front_a(k - 3)
            act_chain.new_group()                 # B phase (Exp/Ln table)
            if k < NCH:
                s1.front_b(k)
            if 0 <= k - 3 < NCH:
                s2.front_b(k - 3)
            if 0 <= k - 2 < NCH:                  # retarded LN stats (B)
                s1.ln_acts(k - 2)
            if 0 <= k - 5 < NCH:
                s2.ln_acts(k - 5)
            if 0 <= k - 1 < NCH:                  # out_proj at PE tail: done
                s1.out_proj(k - 1)                # a full iter before ln_acts
            if 0 <= k - 4 < NCH:
                s2.out_proj(k - 4)
            if k < NCH:
                s1.dve_chain(k)
            if 0 <= k - 3 < NCH:
                s2.dve_chain(k - 3)
            if 0 <= k - 2 < NCH:
                s1.ln_tail(k - 2)
            if 0 <= k - 5 < NCH:
                s2.ln_tail(k - 5)

    nc.finalize()
    return nc


# ----------------------------------------------------------------------------
# entry point
# ----------------------------------------------------------------------------

_NC = {}


def kernel(**inputs):
    global last_exec_time_ns, last_results
    inputs = {k: np.asarray(v) for k, v in inputs.items()}
    weights = prep_weights(inputs)
    x = inputs['x'].astype(np.float32)          # [8, 128, 64, 64]
    b, c, h, w = x.shape
    L = h * w

    a1 = -np.exp(np.asarray(inputs['s1_alog'], np.float32))
    a2 = -np.exp(np.asarray(inputs['s2_alog'], np.float32))
    assert (np.allclose(a1[:, 1], 2 * a1[:, 0], rtol=1e-6) and
            np.allclose(a2[:, 1], 2 * a2[:, 0], rtol=1e-6)), \
        "kernel assumes A2 == 2*A1 (dA1 = dA0^2)"
    if L not in _NC:
        _NC[L] = build_program(L)

    in_maps = [dict(weights, x=np.ascontiguousarray(x[i].reshape(c, L)))
               for i in range(NCORES)]
    res = run_bass_kernel_spmd(
        _NC[L], in_maps, list(range(NCORES)),
        trace=bool(os.environ.get("KBENCH_TRACE")))
    last_exec_time_ns = res.exec_time_ns
    last_results = res
    out = np.stack([np.asarray(res.results[i]['out'], np.float32)
                    .reshape(256, h, w) for i in range(NCORES)])
    return out


# revision 18
# speedup vs baseline: 1.1538x; 1.1538x over previous
"""Trainium2 Bass kernel for nn_Branch_2_36386962932308.

Network (per batch, feature-major planes [channels, L=h*w=4096]):
  stage1: Mamba(d=128, di=128, n=2, r=8, conv4) -> LN
  linear: 128->256 + SiLU   (stage-1 LN affine folded into the linear weight)
  stage2: Mamba(d=256, di=256, n=2, r=16, conv4) -> LN (affine applied on device)

Sharding: data-parallel over batch, one batch element per NeuronCore (8 cores).

v3 — chunk-interleaved software pipeline:
  - stage1(chunk k) and stage2(chunk k-2) are processed in the same pipeline
    iteration, so stage2's Tensor/ACT work fills stage1's latency gaps.
  - ACT table phases per iteration: A = [SiLUs], B = [dt chains] + the
    PREVIOUS chunk's LayerNorm input ops (yp->SBUF Identity + Square, which
    are table-neutral).  Nothing late ever heads the ACT queue, so the
    in-order engine queues never head-of-line block.
  - LayerNorm rstd = 1/sqrt(var+eps) computed WITHOUT ACT Ln/Exp: bit-trick
    seed (DVE int ALU) + two Newton steps on the (otherwise idle) GpSimd.
  - out_proj groups are packed as views into [SUB,512] PSUM bank tiles
    (s2: 2x256 halves, s1: 4x128 quarters); yp leaves PSUM via one ACT
    Identity (accum gives sum -> mean), so PSUM never backs up the PE.
  - normalize = single DVE tensor_scalar in 4x mode (bf16 SBUF operands).
  - causal depthwise conv folded into in_proj (4 shifted matmuls); B/C rows
    replicated via DMA row-flatten + Pool partition_broadcast; dA1 = dA0^2
    on Pool; bf16 on all elementwise-heavy paths.
  - LN outputs return to feature-major via PE transpose (the XBAR DMA
    transpose corrupts even lanes on this platform).

Self-contained: hardcodes all shapes; needs only concourse + numpy at runtime.
"""

import os
from contextlib import ExitStack

import numpy as np

import concourse.bass as bass
import concourse.bacc as bacc
import concourse.mybir as mybir
import concourse.tile as tile
from concourse.bass_utils import run_bass_kernel_spmd

F32 = mybir.dt.float32
BF16 = mybir.dt.bfloat16
I32 = mybir.dt.int32
AF = mybir.ActivationFunctionType
ALU = mybir.AluOpType

NCORES = 8
LN_EPS = 1e-5
CH = 512          # pipeline column chunk (one PSUM bank at fp32)
SUB = 128         # out_proj / LN subchunk (time-major tile height)
MAGIC = 0x5F3759DF

last_exec_time_ns = None
last_results = None


def _patch_act_tables():
    """Make natural_log_exp_and_others the only table set containing Exp and
    Ln, so bacc's table-load placement keeps one set resident through each
    B phase instead of swapping between exp_and_others and natural_log on
    every Exp<->Ln transition (~2.7us per swap)."""
    import functools
    import concourse.hw_specs as hw_specs
    if getattr(hw_specs.get_activation_tables, "_lnexp_patched", False):
        return
    orig = hw_specs.get_activation_tables

    @functools.cache
    def patched(arch):
        tables = {k: set(v) for k, v in orig(arch).items()}
        for name, fns in tables.items():
            if name != 'natural_log_exp_and_others':
                fns.discard(AF.Exp)
                fns.discard(AF.Ln)
        return tables

    patched._lnexp_patched = True
    hw_specs.get_activation_tables = patched
    bacc.get_activation_tables = patched


_patch_act_tables()


# ----------------------------------------------------------------------------
# host-side weight preparation
# ----------------------------------------------------------------------------

def _prep_stage(p, d, di, r):
    win = np.asarray(p['win'], np.float32)
    b_in = np.asarray(p['bin'], np.float32)
    cw = np.asarray(p['cw'], np.float32)        # [di, 1, 4]
    cb = np.asarray(p['cb'], np.float32)
    wx = np.asarray(p['wx'], np.float32)        # [r+4, di]
    wdt = np.asarray(p['wdt'], np.float32)      # [di, r]
    bdt = np.asarray(p['bdt'], np.float32)
    alog = np.asarray(p['alog'], np.float32)    # [di, 2]
    dd = np.asarray(p['dd'], np.float32)
    wout = np.asarray(p['wout'], np.float32)    # [dout, di]

    winx, winz = win[:di], win[di:]
    w_k = np.stack([np.ascontiguousarray((cw[:, 0, k:k + 1] * winx).T)
                    for k in range(4)])          # [4, d, di]
    wz = np.ascontiguousarray(winz.T)            # [d, di]
    wxT = np.ascontiguousarray(wx.T)             # [di, r+4]
    wdtT = np.ascontiguousarray(wdt.T)           # [r, di]
    # Center wout's output columns so yp = yg @ wout is mean-free by
    # construction (LayerNorm mean-subtraction folded into the weights).
    # Iterate against bf16 rounding so the bf16-stored weights still have
    # (near-)zero column mean.
    bf = mybir.dt.np(BF16)
    wc = wout - wout.mean(0, keepdims=True)
    for _ in range(3):
        wc = wc - wc.astype(bf).astype(np.float32).mean(0, keepdims=True)
    woutT = np.ascontiguousarray(wc.T)           # [di, dout]

    S = cw[:, 0, :].sum(1)
    silu_bias = cb + S * b_in[:di]
    bz = b_in[di:]
    A = -np.exp(alog)                            # [di, 2] (negative)
    corr = np.stack([-(cw[:, 0, :3 - t].sum(1)) * b_in[:di] for t in range(3)], 1)
    cols = [silu_bias, bz, bdt, A[:, 0], A[:, 1], dd,
            corr[:, 0], corr[:, 1], corr[:, 2]]
    return w_k, wz, wxT, wdtT, woutT, np.stack(cols, 1).astype(np.float32)


def prep_weights(inputs):
    s1 = {k[3:]: inputs[k] for k in inputs if k.startswith('s1_')}
    s2 = {k[3:]: inputs[k] for k in inputs if k.startswith('s2_')}
    w1k, w1z, wx1, wdt1, wout1, cols1 = _prep_stage(s1, 128, 128, 8)
    w2k, w2z, wx2, wdt2, wout2, cols2 = _prep_stage(s2, 256, 256, 16)
    lnw2 = np.asarray(s2['lnw'], np.float32)
    lnb2 = np.asarray(s2['lnb'], np.float32)
    cols2 = np.concatenate([cols2, lnw2[:, None], lnb2[:, None]], 1)
    cols2 = np.ascontiguousarray(cols2, dtype=np.float32)

    bfdt = mybir.dt.np(BF16)
    lin_w = np.asarray(inputs['lin_w'], np.float32)
    lin_b = np.asarray(inputs['lin_b'], np.float32)
    lnw1 = np.asarray(s1['lnw'], np.float32)
    lnb1 = np.asarray(s1['lnb'], np.float32)
    linw = np.ascontiguousarray((lin_w * lnw1[None, :]).T)
    linb = (lin_w @ lnb1 + lin_b).astype(np.float32)[:, None]

    return {
        'idenb': np.eye(128, dtype=np.float32).astype(bfdt),
        'w1k': w1k, 'w1z': w1z, 'wx1': wx1.astype(bfdt),
        'wdt1': wdt1.astype(bfdt),
        'wout1': wout1.astype(bfdt), 'cols1': cols1,
        'w2k': w2k.astype(bfdt), 'w2z': w2z.astype(bfdt),
        'wx2': wx2.astype(bfdt), 'wdt2': wdt2.astype(bfdt),
        'wout2': wout2.astype(bfdt), 'cols2': cols2,
        'linw': linw.astype(bfdt), 'linb': linb,
    }


# ----------------------------------------------------------------------------
# device program
# ----------------------------------------------------------------------------

F32R = mybir.dt.float32r


def _tile(pool, shape, dtype, tag, bufs=None):
    return pool.tile(shape, dtype, tag=tag, name=tag, bufs=bufs)


def _mmr(nc, out, lhsT, rhs, **kw):
    """fp32 matmul via float32r bitcast: single-pass on the PE."""
    nc.tensor.matmul(out, lhsT.bitcast(F32R), rhs.bitcast(F32R), **kw)


def _mmb(nc, out, lhsT, rhs, **kw):
    """all-bf16 matmul (full PE rate at any N)."""
    nc.tensor.matmul(out, lhsT, rhs, **kw)


class _ActChain:
    """Groups ACT instructions into table-set phases separated by no-op
    barrier instructions, so the scheduler can reorder freely within a phase
    (same table set) but cannot interleave phases (which would make bacc
    insert a ~2.7us ACT table load per out-of-phase function switch)."""

    def __init__(self, nc, bar_tile):
        self.nc = nc
        self.bar_tile = bar_tile
        self.group = []
        self.barrier = None

    def new_group(self):
        from concourse.tile_rust import add_dep_helper
        if not self.group:
            return
        bar = self.nc.scalar.activation(self.bar_tile[:], self.bar_tile[:],
                                        AF.Identity)
        barc = bar.ins if hasattr(bar, 'ins') else bar
        for op in self.group:
            add_dep_helper(barc, op, sync=False, reason="act phase barrier")
        self.barrier = barc
        self.group = []

    def __call__(self, *args, **kwargs):
        from concourse.tile_rust import add_dep_helper
        inst = self.nc.scalar.activation(*args, **kwargs)
        cur = inst.ins if hasattr(inst, 'ins') else inst
        if self.barrier is not None:
            add_dep_helper(cur, self.barrier, sync=False,
                           reason="act phase order")
        self.group.append(cur)
        return inst


class _Stage:
    """One Mamba stage's per-chunk pipeline pieces."""

    def __init__(self, nc, act, pools, P_in, P, r, dout, in_planes,
                 wk, wz, wx, wdt, wout, cols, mm_in, ps_y, emit, flush,
                 pfx=""):
        self.nc = nc
        self.act = act
        self.pools = pools
        self.pfx = pfx
        self.P_in, self.P, self.r, self.dout = P_in, P, r, dout
        self.in_planes = in_planes
        self.wk, self.wz, self.wx, self.wdt, self.wout, self.cols = \
            wk, wz, wx, wdt, wout, cols
        self.mm_in = mm_in
        self.ps_y = ps_y
        self.emit = emit
        self.flush = flush
        self.hs_prev = [[None] * P, [None] * P]
        self.st = {}                  # chunk k -> dict of live tiles

    # ---- A phase: in_proj (conv folded) + z + SiLU ----------------------
    def front_a(self, k):
        nc, act = self.nc, self.act
        ps_mm = self.pools['mm']
        c0 = k * CH
        s = {}
        xc = [None] * self.P
        sz = [None] * self.P
        for mi in range(self.P):
            ms = slice(mi * 128, (mi + 1) * 128)
            xc_ps = _tile(ps_mm, [128, CH], F32, "mm", 3)
            nmm = 4 * self.P_in
            i = 0
            for kk in range(4):
                for kt in range(self.P_in):
                    self.mm_in(nc, xc_ps[:], self.wk[kk][kt][:, ms],
                               self.in_planes[kt][:, c0 + kk: c0 + kk + CH],
                               start=(i == 0), stop=(i == nmm - 1))
                    i += 1
            if c0 == 0:
                nc.vector.tensor_add(xc_ps[:, 0:3], xc_ps[:, 0:3],
                                     self.cols[mi][:, 6:9])
            t_xc = _tile(self.pools['sb'], [128, CH], BF16,
                         self.pfx + "xc", 2 * self.P + 2)
            act(t_xc[:], xc_ps[:], AF.Silu, bias=self.cols[mi][:, 0:1])
            xc[mi] = t_xc

            z_ps = _tile(ps_mm, [128, CH], F32, "mm", 3)
            for kt in range(self.P_in):
                self.mm_in(nc, z_ps[:], self.wz[kt][:, ms],
                           self.in_planes[kt][:, c0 + 3: c0 + 3 + CH],
                           start=(kt == 0), stop=(kt == self.P_in - 1))
            t_sz = _tile(self.pools['sb'], [128, CH], BF16,
                         self.pfx + "sz", 2 * self.P + 2)
            act(t_sz[:], z_ps[:], AF.Silu, bias=self.cols[mi][:, 1:2])
            sz[mi] = t_sz
        s['xc'], s['sz'] = xc, sz
        self.st[k] = s

    # ---- B phase: wx/dt projections + dt chain --------------------------
    def front_b(self, k):
        nc, act = self.nc, self.act
        sb, ps_mm = self.pools['sb'], self.pools['mm']
        rw = self.r + 4
        s = self.st[k]
        xdbl_ps = _tile(ps_mm, [128, CH], F32, "mm", 3)
        for kt in range(self.P_in):
            _mmb(nc, xdbl_ps[:rw, :], self.wx[kt][:], s['xc'][kt][:],
                 start=(kt == 0), stop=(kt == self.P_in - 1))
        xdbl = _tile(sb, [rw, CH], BF16, self.pfx + "xdbl", 2)
        act(xdbl[:], xdbl_ps[:rw, :], AF.Identity)
        s['xdbl'] = xdbl

        dt_sb = []
        dA_sb = [[None] * self.P, [None] * self.P]
        for mi in range(self.P):
            ms = slice(mi * 128, (mi + 1) * 128)
            dt_ps = _tile(ps_mm, [128, CH], F32, "mm", 3)
            _mmb(nc, dt_ps[:], self.wdt[:, ms], xdbl[:self.r, :])
            t_e = _tile(sb, [128, CH], F32, "scr1", 3)
            act(t_e[:], dt_ps[:], AF.Exp, bias=self.cols[mi][:, 2:3])
            t_dt = _tile(sb, [128, CH], BF16, self.pfx + "dt", self.P + 2)
            act(t_dt[:], t_e[:], AF.Ln, bias=1.0)
            dt_sb.append(t_dt)
            t_dA0 = _tile(sb, [128, CH], F32, self.pfx + "dA0", self.P + 1)
            act(t_dA0[:], t_dt[:], AF.Exp, scale=self.cols[mi][:, 3:4])
            dA_sb[0][mi] = t_dA0
            t_dA1 = _tile(sb, [128, CH], F32, self.pfx + "dA1", self.P + 1)
            nc.gpsimd.tensor_mul(t_dA1[:], t_dA0[:], t_dA0[:])
            dA_sb[1][mi] = t_dA1
        s['dt'], s['dA'] = dt_sb, dA_sb
        rows = _tile(sb, [1, 4 * CH], BF16, "rows", 2)
        nc.scalar.dma_start(rows[:], xdbl[self.r:self.r + 4, :])
        rep_all = _tile(sb, [128, 4 * CH], BF16, "repall", 3)
        nc.gpsimd.partition_broadcast(rep_all[:], rows[:])
        s['rep'] = rep_all

    # ---- DVE: gate/scan chain -------------------------------------------
    def dve_chain(self, k):
        nc = self.nc
        sb = self.pools['sb']
        c0 = k * CH
        s = self.st[k]
        rep_all = s.pop('rep')
        rep = [rep_all[:, j * CH:(j + 1) * CH] for j in range(4)]
        hs = [[None] * self.P, [None] * self.P]
        for mi in range(self.P):
            t_u = _tile(sb, [128, CH], BF16, self.pfx + "scr2", 3)
            nc.vector.tensor_mul(t_u[:], s['dt'][mi][:], s['xc'][mi][:])
            for n in range(2):
                t_dbu = _tile(sb, [128, CH], BF16, self.pfx + f"dbu{n}", 2)
                nc.vector.tensor_mul(t_dbu[:], t_u[:], rep[n])
                t_hs = _tile(sb, [128, CH], BF16, self.pfx + f"hs{n}",
                             2 * self.P + 1)
                init = (0.0 if c0 == 0 else
                        self.hs_prev[n][mi][:, CH - 1:CH])
                nc.vector.tensor_tensor_scan(
                    t_hs[:], s['dA'][n][mi][:], t_dbu[:], init,
                    ALU.mult, ALU.add)
                hs[n][mi] = t_hs
                self.hs_prev[n][mi] = t_hs

        yg = []
        for mi in range(self.P):
            t_m0 = _tile(sb, [128, CH], BF16, self.pfx + "m0", 2)
            nc.vector.tensor_mul(t_m0[:], hs[0][mi][:], rep[2])
            t_y = _tile(sb, [128, CH], BF16, self.pfx + "y", 2)
            nc.vector.tensor_mul(t_y[:], hs[1][mi][:], rep[3])
            nc.vector.tensor_add(t_y[:], t_y[:], t_m0[:])
            t_dx = _tile(sb, [128, CH], BF16, self.pfx + "dx", 2)
            nc.vector.tensor_scalar(t_dx[:], s['xc'][mi][:],
                                    self.cols[mi][:, 5:6], None,
                                    ALU.mult, ALU.bypass)
            nc.vector.tensor_add(t_y[:], t_y[:], t_dx[:])
            t_yg = _tile(sb, [128, CH], BF16, self.pfx + "yg",
                         2 * self.P + 3)
            nc.vector.tensor_mul(t_yg[:], t_y[:], s['sz'][mi][:])
            yg.append(t_yg)
        s['yg'] = yg

    # ---- PE: out_proj into packed PSUM bank tiles -----------------------
    def out_proj(self, k):
        nc = self.nc
        s = self.st[k]
        views = []
        n_banks = (4 * self.dout) // 512       # s1: 1, s2: 2
        per_bank = 512 // self.dout            # s1: 4, s2: 2
        for b in range(n_banks):
            bank = _tile(self.ps_y, [SUB, 512], F32, "yb", 2)
            for h in range(per_bank):
                g = b * per_bank + h
                cs = slice(g * SUB, (g + 1) * SUB)
                dst = bank[:, h * self.dout:(h + 1) * self.dout]
                for mi in range(self.P):
                    nc.tensor.matmul(dst, s['yg'][mi][:, cs],
                                     self.wout[mi][:],
                                     start=(mi == 0), stop=(mi == self.P - 1))
                views.append(dst)
        s['yp'] = views

    # ---- B phase (retarded): yp -> SBUF + LN stats ----------------------
    def ln_acts(self, k):
        nc = self.nc
        sb = self.pools['sb']
        s = self.st[k]
        ssq4 = _tile(sb, [SUB, 4], F32, self.pfx + "ssq4", 2)
        ypsb = []
        for g in range(4):
            t_y = _tile(sb, [SUB, self.dout], BF16, self.pfx + "ypsb", 8)
            nc.scalar.activation(t_y[:], s['yp'][g], AF.Identity)
            scr = _tile(sb, [SUB, self.dout], BF16, self.pfx + "scr", 2)
            nc.scalar.activation(scr[:], t_y[:], AF.Square,
                                 accum_out=ssq4[:, g:g + 1])
            ypsb.append(t_y)
        s['ypsb'], s['ssq4'] = ypsb, ssq4

    # ---- Pool/DVE: rstd chain + normalize + emit ------------------------
    def ln_tail(self, k):
        nc = self.nc
        sb = self.pools['sb']
        s = self.st.pop(k)
        c0 = k * CH
        dout = self.dout
        veps = _tile(sb, [SUB, 4], F32, self.pfx + "veps", 2)
        nc.gpsimd.tensor_scalar(veps[:], s['ssq4'][:], 1.0 / dout, LN_EPS,
                                ALU.mult, ALU.add)
        i1 = _tile(sb, [SUB, 4], I32, self.pfx + "i1", 2)
        nc.vector.tensor_scalar(i1[:], veps[:].bitcast(I32), 1, None,
                                ALU.arith_shift_right, ALU.bypass)
        nc.vector.tensor_scalar(i1[:], i1[:], 0xFFFFFFFF, None,
                                ALU.bitwise_xor, ALU.bypass)
        nc.vector.tensor_scalar(i1[:], i1[:], MAGIC + 1, None,
                                ALU.add, ALU.bypass)
        r0 = i1[:].bitcast(F32)
        vh = _tile(sb, [SUB, 4], F32, self.pfx + "vh", 2)
        nc.gpsimd.tensor_scalar(vh[:], veps[:], -0.5, None,
                                ALU.mult, ALU.bypass)
        for it in range(2):
            t_nr = _tile(sb, [SUB, 4], F32, self.pfx + f"nr{it}", 2)
            nc.gpsimd.tensor_mul(t_nr[:], r0, vh[:])
            nc.gpsimd.tensor_mul(t_nr[:], t_nr[:], r0)
            nc.gpsimd.tensor_scalar(t_nr[:], t_nr[:], 1.5, None,
                                    ALU.add, ALU.bypass)
            r1 = _tile(sb, [SUB, 4], F32, self.pfx + f"rst{it}", 2)
            nc.gpsimd.tensor_mul(r1[:], r0, t_nr[:])
            r0 = r1[:]
        for g in range(4):
            tn = _tile(sb, [SUB, dout], BF16, self.pfx + "tn", 2)
            nc.vector.tensor_scalar(tn[:], s['ypsb'][g][:],
                                    r0[:, g:g + 1], None,
                                    ALU.mult, ALU.bypass)
            self.emit(tn, c0, g)
        self.flush(c0)


def build_program(L=4096):
    nc = bacc.Bacc()
    dp = nc.declare_dram_parameter
    x_d = dp("x", [128, L], F32R, isOutput=False)
    w1k_d = dp("w1k", [4, 128, 128], F32R, isOutput=False)
    w1z_d = dp("w1z", [128, 128], F32R, isOutput=False)
    wx1_d = dp("wx1", [128, 12], BF16, isOutput=False)
    wdt1_d = dp("wdt1", [8, 128], BF16, isOutput=False)
    wout1_d = dp("wout1", [128, 128], BF16, isOutput=False)
    cols1_d = dp("cols1", [128, 9], F32, isOutput=False)
    w2k_d = dp("w2k", [4, 256, 256], BF16, isOutput=False)
    w2z_d = dp("w2z", [256, 256], BF16, isOutput=False)
    wx2_d = dp("wx2", [256, 20], BF16, isOutput=False)
    wdt2_d = dp("wdt2", [16, 256], BF16, isOutput=False)
    wout2_d = dp("wout2", [256, 256], BF16, isOutput=False)
    cols2_d = dp("cols2", [256, 11], F32, isOutput=False)
    linw_d = dp("linw", [128, 256], BF16, isOutput=False)
    linb_d = dp("linb", [256, 1], F32, isOutput=False)
    iden_d = dp("idenb", [128, 128], BF16, isOutput=False)
    out_d = dp("out", [256, L], BF16, isOutput=True)

    dma = nc.sync.dma_start
    NCH = L // CH

    with tile.TileContext(nc) as tc, ExitStack() as ctx:
        consts = ctx.enter_context(tc.tile_pool(name="consts", bufs=1))
        planes = ctx.enter_context(tc.tile_pool(name="planes", bufs=1))
        sb = ctx.enter_context(tc.tile_pool(name="sb", bufs=2))
        ps_mm = ctx.enter_context(
            tc.tile_pool(name="psmm", bufs=3, space=bass.MemorySpace.PSUM))
        ps_y1 = ctx.enter_context(
            tc.tile_pool(name="psy1", bufs=2, space=bass.MemorySpace.PSUM))
        ps_y2 = ctx.enter_context(
            tc.tile_pool(name="psy2", bufs=2, space=bass.MemorySpace.PSUM))
        ps_tf = ctx.enter_context(
            tc.tile_pool(name="pstf", bufs=1, space=bass.MemorySpace.PSUM))
        pools = {'sb': sb, 'mm': ps_mm}

        xpad = planes.tile([128, L + 3], F32R, tag="xpad", name="xpad")
        nc.gpsimd.memset(xpad[:, 0:3].bitcast(F32), 0.0)
        for s0 in range(0, L, 2048):
            dma(xpad[:, 3 + s0: 3 + s0 + 2048], x_d[:, s0:s0 + 2048])

        _ld = [0]

        def load(dram_ap, shape, dtype=F32R):
            _ld[0] += 1
            t = consts.tile(shape, dtype, tag=f"w{_ld[0]}", name=f"w{_ld[0]}")
            dma(t[:], dram_ap)
            return t

        w1k_sb = [[load(w1k_d[k], [128, 128])] for k in range(4)]
        w1z_sb = [load(w1z_d[:], [128, 128])]
        wx1_sb = [load(wx1_d[:], [128, 12], BF16)]
        wdt1_sb = load(wdt1_d[:], [8, 128], BF16)
        wout1_sb = [load(wout1_d[:], [128, 128], BF16)]
        cols1_sb = [load(cols1_d[:], [128, 9], F32)]
        w2k_sb = [[load(w2k_d[k, kt * 128:(kt + 1) * 128], [128, 256], BF16)
                   for kt in range(2)] for k in range(4)]
        w2z_sb = [load(w2z_d[kt * 128:(kt + 1) * 128], [128, 256], BF16)
                  for kt in range(2)]
        wx2_sb = [load(wx2_d[kt * 128:(kt + 1) * 128], [128, 20], BF16)
                  for kt in range(2)]
        wdt2_sb = load(wdt2_d[:], [16, 256], BF16)
        wout2_sb = [load(wout2_d[kt * 128:(kt + 1) * 128], [128, 256], BF16)
                    for kt in range(2)]
        cols2_sb = [load(cols2_d[kt * 128:(kt + 1) * 128], [128, 11], F32)
                    for kt in range(2)]
        linw_sb = load(linw_d[:], [128, 256], BF16)
        linb_sb = [load(linb_d[kt * 128:(kt + 1) * 128], [128, 1], F32)
                   for kt in range(2)]

        bar_tile = consts.tile([1, 1], F32, tag="actbar", name="actbar")
        nc.gpsimd.memset(bar_tile[:], 0.0)
        act_chain = _ActChain(nc, bar_tile)
        ident = consts.tile([128, 128], BF16, tag="ident", name="ident")
        dma(ident[:], iden_d[:])

        t1n = planes.tile([128, L], BF16, tag="t1n", name="t1n")
        t2pad = [planes.tile([128, L + 3], BF16, tag=f"t2pad{mi}",
                             name=f"t2pad{mi}") for mi in range(2)]
        for mi in range(2):
            nc.gpsimd.memset(t2pad[mi][:, 0:3], 0.0)

        # ---- emits ----
        def emit1(tn, c0, g):
            tf = _tile(ps_tf, [128, 2 * SUB], BF16, "tf", 1)
            nc.tensor.transpose(tf[:, :SUB], tn[:], ident[:])
            nc.vector.tensor_scalar(
                t1n[:, c0 + g * SUB: c0 + (g + 1) * SUB], tf[:, :SUB],
                1.0, None, ALU.mult, ALU.bypass)

        of_buf = [None, None]

        def emit2(tn, c0, g):
            tf = _tile(ps_tf, [128, 2 * SUB], BF16, "tf", 1)
            for ct in range(2):
                if g == 0:
                    of_buf[ct] = _tile(sb, [128, CH], BF16, f"of{ct}", 2)
                nc.tensor.transpose(tf[:, ct * SUB:(ct + 1) * SUB],
                                    tn[:, ct * 128:(ct + 1) * 128],
                                    ident[:])
                nc.vector.tensor_scalar(
                    of_buf[ct][:, g * SUB:(g + 1) * SUB],
                    tf[:, ct * SUB:(ct + 1) * SUB],
                    cols2_sb[ct][:, 9:10], cols2_sb[ct][:, 10:11],
                    ALU.mult, ALU.add)

        def flush2(c0):
            for ct in range(2):
                dma(out_d[ct * 128:(ct + 1) * 128, c0:c0 + CH],
                    of_buf[ct][:])

        s1 = _Stage(nc, act_chain, pools, 1, 1, 8, 128, [xpad],
                    w1k_sb, w1z_sb, wx1_sb, wdt1_sb, wout1_sb, cols1_sb,
                    _mmr, ps_y1, emit1, lambda c0: None, pfx="a")
        s2 = _Stage(nc, act_chain, pools, 2, 2, 16, 256, t2pad,
                    w2k_sb, w2z_sb, wx2_sb, wdt2_sb, wout2_sb, cols2_sb,
                    _mmb, ps_y2, emit2, flush2, pfx="b")

        def linear_a(k):
            c0 = k * CH
            for mi in range(2):
                ms = slice(mi * 128, (mi + 1) * 128)
                lp = _tile(ps_mm, [128, CH], F32, "mm", 3)
                _mmb(nc, lp[:], linw_sb[:, ms], t1n[:, c0:c0 + CH])
                act_chain(t2pad[mi][:, 3 + c0: 3 + c0 + CH], lp[:],
                          AF.Silu, bias=linb_sb[mi][:, 0:1])

        # ---- the interleaved chunk pipeline ----
        # iter k: s1 front(k); s2 front(k-3); LN stats retarded by TWO
        # chunks (s1: k-2, s2: k-5) so every ACT-chained op is at least one
        # full iteration old when the in-order ACT queue reaches it.
        # out_proj runs at the PE stream front for the same reason.
        for k in range(NCH + 6):
            act_chain.new_group()                 # A phase (Silu table)
            if 0 <= k - 3 < NCH:
                linear_a(k - 3)
            if k < NCH:
                s1.front_a(k)
            if 0 <= k - 3 < NCH:
                s2.front_a(k - 3)
            act_chain.new_group()                 # B phase (Exp/Ln table)
            if k < NCH:
                s1.front_b(k)
            if 0 <= k - 3 < NCH:
                s2.front_b(k - 3)
            if 0 <= k - 2 < NCH:                  # retarded LN stats (B)
                s1.ln_acts(k - 2)
            if 0 <= k - 5 < NCH:
                s2.ln_acts(k - 5)
            if 0 <= k - 1 < NCH:                  # out_proj at PE tail: done
                s1.out_proj(k - 1)                # a full iter before ln_acts
            if 0 <= k - 4 < NCH:
                s2.out_proj(k - 4)
            if k < NCH:
                s1.dve_chain(k)
            if 0 <= k - 3 < NCH:
                s2.dve_chain(k - 3)
            if 0 <= k - 2 < NCH:
                s1.ln_tail(k - 2)
            if 0 <= k - 5 < NCH:
                s2.ln_tail(k - 5)

    nc.finalize()
    return nc


# ----------------------------------------------------------------------------
# entry point
# ----------------------------------------------------------------------------

_NC = {}


def kernel(**inputs):
    global last_exec_time_ns, last_results
    inputs = {k: np.asarray(v) for k, v in inputs.items()}
    weights = prep_weights(inputs)
    x = inputs['x'].astype(np.float32)          # [8, 128, 64, 64]
    b, c, h, w = x.shape
    L = h * w

    a1 = -np.exp(np.asarray(inputs['s1_alog'], np.float32))
    a2 = -np.exp(np.asarray(inputs['s2_alog'], np.float32))
    assert (np.allclose(a1[:, 1], 2 * a1[:, 0], rtol=1e-6) and
            np.allclose(a2[:, 1], 2 * a2[:, 0], rtol=1e-6)), \
        "kernel assumes A2 == 2*A1 (dA1 = dA0^2)"
    if L not in _NC:
        _NC[L] = build_program(L)

    in_maps = [dict(weights, x=np.ascontiguousarray(x[i].reshape(c, L)))
               for i in range(NCORES)]
    res = run_bass_kernel_spmd(
        _NC[L], in_maps, list(range(NCORES)),
        trace=bool(os.environ.get("KBENCH_TRACE")))
    last_exec_time_ns = res.exec_time_ns
    last_results = res
    out = np.stack([np.asarray(res.results[i]['out'], np.float32)
                    .reshape(256, h, w) for i in range(NCORES)])
    return out


# revision 19
# speedup vs baseline: 1.1567x; 1.0026x over previous
"""Trainium2 Bass kernel for nn_Branch_2_36386962932308.

Network (per batch, feature-major planes [channels, L=h*w=4096]):
  stage1: Mamba(d=128, di=128, n=2, r=8, conv4) -> LN
  linear: 128->256 + SiLU   (stage-1 LN affine folded into the linear weight)
  stage2: Mamba(d=256, di=256, n=2, r=16, conv4) -> LN (affine applied on device)

Sharding: data-parallel over batch, one batch element per NeuronCore (8 cores).

v3 — chunk-interleaved software pipeline:
  - stage1(chunk k) and stage2(chunk k-2) are processed in the same pipeline
    iteration, so stage2's Tensor/ACT work fills stage1's latency gaps.
  - ACT table phases per iteration: A = [SiLUs], B = [dt chains] + the
    PREVIOUS chunk's LayerNorm input ops (yp->SBUF Identity + Square, which
    are table-neutral).  Nothing late ever heads the ACT queue, so the
    in-order engine queues never head-of-line block.
  - LayerNorm rstd = 1/sqrt(var+eps) computed WITHOUT ACT Ln/Exp: bit-trick
    seed (DVE int ALU) + two Newton steps on the (otherwise idle) GpSimd.
  - out_proj groups are packed as views into [SUB,512] PSUM bank tiles
    (s2: 2x256 halves, s1: 4x128 quarters); yp leaves PSUM via one ACT
    Identity (accum gives sum -> mean), so PSUM never backs up the PE.
  - normalize = single DVE tensor_scalar in 4x mode (bf16 SBUF operands).
  - causal depthwise conv folded into in_proj (4 shifted matmuls); B/C rows
    replicated via DMA row-flatten + Pool partition_broadcast; dA1 = dA0^2
    on Pool; bf16 on all elementwise-heavy paths.
  - LN outputs return to feature-major via PE transpose (the XBAR DMA
    transpose corrupts even lanes on this platform).

Self-contained: hardcodes all shapes; needs only concourse + numpy at runtime.
"""

import os
from contextlib import ExitStack

import numpy as np

import concourse.bass as bass
import concourse.bacc as bacc
import concourse.mybir as mybir
import concourse.tile as tile
from concourse.bass_utils import run_bass_kernel_spmd

F32 = mybir.dt.float32
BF16 = mybir.dt.bfloat16
I32 = mybir.dt.int32
AF = mybir.ActivationFunctionType
ALU = mybir.AluOpType

NCORES = 8
LN_EPS = 1e-5
CH = 512          # pipeline column chunk (one PSUM bank at fp32)
SUB = 128         # out_proj / LN subchunk (time-major tile height)
MAGIC = 0x5F3759DF

last_exec_time_ns = None
last_results = None


def _patch_act_tables():
    """Make natural_log_exp_and_others the only table set containing Exp and
    Ln, so bacc's table-load placement keeps one set resident through each
    B phase instead of swapping between exp_and_others and natural_log on
    every Exp<->Ln transition (~2.7us per swap)."""
    import functools
    import concourse.hw_specs as hw_specs
    if getattr(hw_specs.get_activation_tables, "_lnexp_patched", False):
        return
    orig = hw_specs.get_activation_tables

    @functools.cache
    def patched(arch):
        tables = {k: set(v) for k, v in orig(arch).items()}
        for name, fns in tables.items():
            if name != 'natural_log_exp_and_others':
                fns.discard(AF.Exp)
                fns.discard(AF.Ln)
        return tables

    patched._lnexp_patched = True
    hw_specs.get_activation_tables = patched
    bacc.get_activation_tables = patched


_patch_act_tables()


# ----------------------------------------------------------------------------
# host-side weight preparation
# ----------------------------------------------------------------------------

def _prep_stage(p, d, di, r):
    win = np.asarray(p['win'], np.float32)
    b_in = np.asarray(p['bin'], np.float32)
    cw = np.asarray(p['cw'], np.float32)        # [di, 1, 4]
    cb = np.asarray(p['cb'], np.float32)
    wx = np.asarray(p['wx'], np.float32)        # [r+4, di]
    wdt = np.asarray(p['wdt'], np.float32)      # [di, r]
    bdt = np.asarray(p['bdt'], np.float32)
    alog = np.asarray(p['alog'], np.float32)    # [di, 2]
    dd = np.asarray(p['dd'], np.float32)
    wout = np.asarray(p['wout'], np.float32)    # [dout, di]

    winx, winz = win[:di], win[di:]
    w_k = np.stack([np.ascontiguousarray((cw[:, 0, k:k + 1] * winx).T)
                    for k in range(4)])          # [4, d, di]
    wz = np.ascontiguousarray(winz.T)            # [d, di]
    wxT = np.ascontiguousarray(wx.T)             # [di, r+4]
    wdtT = np.ascontiguousarray(wdt.T)           # [r, di]
    # Center wout's output columns so yp = yg @ wout is mean-free by
    # construction (LayerNorm mean-subtraction folded into the weights).
    # Iterate against bf16 rounding so the bf16-stored weights still have
    # (near-)zero column mean.
    bf = mybir.dt.np(BF16)
    wc = wout - wout.mean(0, keepdims=True)
    for _ in range(3):
        wc = wc - wc.astype(bf).astype(np.float32).mean(0, keepdims=True)
    woutT = np.ascontiguousarray(wc.T)           # [di, dout]

    S = cw[:, 0, :].sum(1)
    silu_bias = cb + S * b_in[:di]
    bz = b_in[di:]
    A = -np.exp(alog)                            # [di, 2] (negative)
    corr = np.stack([-(cw[:, 0, :3 - t].sum(1)) * b_in[:di] for t in range(3)], 1)
    cols = [silu_bias, bz, bdt, A[:, 0], A[:, 1], dd,
            corr[:, 0], corr[:, 1], corr[:, 2]]
    return w_k, wz, wxT, wdtT, woutT, np.stack(cols, 1).astype(np.float32)


def prep_weights(inputs):
    s1 = {k[3:]: inputs[k] for k in inputs if k.startswith('s1_')}
    s2 = {k[3:]: inputs[k] for k in inputs if k.startswith('s2_')}
    w1k, w1z, wx1, wdt1, wout1, cols1 = _prep_stage(s1, 128, 128, 8)
    w2k, w2z, wx2, wdt2, wout2, cols2 = _prep_stage(s2, 256, 256, 16)
    lnw2 = np.asarray(s2['lnw'], np.float32)
    lnb2 = np.asarray(s2['lnb'], np.float32)
    cols2 = np.concatenate([cols2, lnw2[:, None], lnb2[:, None]], 1)
    cols2 = np.ascontiguousarray(cols2, dtype=np.float32)

    bfdt = mybir.dt.np(BF16)
    lin_w = np.asarray(inputs['lin_w'], np.float32)
    lin_b = np.asarray(inputs['lin_b'], np.float32)
    lnw1 = np.asarray(s1['lnw'], np.float32)
    lnb1 = np.asarray(s1['lnb'], np.float32)
    linw = np.ascontiguousarray((lin_w * lnw1[None, :]).T)
    linb = (lin_w @ lnb1 + lin_b).astype(np.float32)[:, None]

    return {
        'idenb': np.eye(128, dtype=np.float32).astype(bfdt),
        'w1k': w1k, 'w1z': w1z, 'wx1': wx1.astype(bfdt),
        'wdt1': wdt1.astype(bfdt),
        'wout1': wout1.astype(bfdt), 'cols1': cols1,
        'w2k': w2k.astype(bfdt), 'w2z': w2z.astype(bfdt),
        'wx2': wx2.astype(bfdt), 'wdt2': wdt2.astype(bfdt),
        'wout2': wout2.astype(bfdt), 'cols2': cols2,
        'linw': linw.astype(bfdt), 'linb': linb,
    }


# ----------------------------------------------------------------------------
# device program
# ----------------------------------------------------------------------------

F32R = mybir.dt.float32r


def _tile(pool, shape, dtype, tag, bufs=None):
    return pool.tile(shape, dtype, tag=tag, name=tag, bufs=bufs)


def _mmr(nc, out, lhsT, rhs, **kw):
    """fp32 matmul via float32r bitcast: single-pass on the PE."""
    nc.tensor.matmul(out, lhsT.bitcast(F32R), rhs.bitcast(F32R), **kw)


def _mmb(nc, out, lhsT, rhs, **kw):
    """all-bf16 matmul (full PE rate at any N)."""
    nc.tensor.matmul(out, lhsT, rhs, **kw)


class _ActChain:
    """Groups ACT instructions into table-set phases separated by no-op
    barrier instructions, so the scheduler can reorder freely within a phase
    (same table set) but cannot interleave phases (which would make bacc
    insert a ~2.7us ACT table load per out-of-phase function switch)."""

    def __init__(self, nc, bar_tile):
        self.nc = nc
        self.bar_tile = bar_tile
        self.group = []
        self.barrier = None

    def new_group(self):
        from concourse.tile_rust import add_dep_helper
        if not self.group:
            return
        bar = self.nc.scalar.activation(self.bar_tile[:], self.bar_tile[:],
                                        AF.Identity)
        barc = bar.ins if hasattr(bar, 'ins') else bar
        for op in self.group:
            add_dep_helper(barc, op, sync=False, reason="act phase barrier")
        self.barrier = barc
        self.group = []

    def __call__(self, *args, **kwargs):
        from concourse.tile_rust import add_dep_helper
        inst = self.nc.scalar.activation(*args, **kwargs)
        cur = inst.ins if hasattr(inst, 'ins') else inst
        if self.barrier is not None:
            add_dep_helper(cur, self.barrier, sync=False,
                           reason="act phase order")
        self.group.append(cur)
        return inst


class _Stage:
    """One Mamba stage's per-chunk pipeline pieces."""

    def __init__(self, nc, act, pools, P_in, P, r, dout, in_planes,
                 wk, wz, wx, wdt, wout, cols, mm_in, ps_y, emit, flush,
                 pfx=""):
        self.nc = nc
        self.act = act
        self.pools = pools
        self.pfx = pfx
        self.P_in, self.P, self.r, self.dout = P_in, P, r, dout
        self.in_planes = in_planes
        self.wk, self.wz, self.wx, self.wdt, self.wout, self.cols = \
            wk, wz, wx, wdt, wout, cols
        self.mm_in = mm_in
        self.ps_y = ps_y
        self.emit = emit
        self.flush = flush
        self.hs_prev = [[None] * P, [None] * P]
        self.st = {}                  # chunk k -> dict of live tiles

    # ---- A phase: in_proj (conv folded) + z + SiLU ----------------------
    def front_a(self, k):
        nc, act = self.nc, self.act
        ps_mm = self.pools['mm']
        c0 = k * CH
        s = {}
        xc = [None] * self.P
        sz = [None] * self.P
        for mi in range(self.P):
            ms = slice(mi * 128, (mi + 1) * 128)
            xc_ps = _tile(ps_mm, [128, CH], F32, "mm", 3)
            nmm = 4 * self.P_in
            i = 0
            for kk in range(4):
                for kt in range(self.P_in):
                    self.mm_in(nc, xc_ps[:], self.wk[kk][kt][:, ms],
                               self.in_planes[kt][:, c0 + kk: c0 + kk + CH],
                               start=(i == 0), stop=(i == nmm - 1))
                    i += 1
            if c0 == 0:
                nc.vector.tensor_add(xc_ps[:, 0:3], xc_ps[:, 0:3],
                                     self.cols[mi][:, 6:9])
            t_xc = _tile(self.pools['sb'], [128, CH], BF16,
                         self.pfx + "xc", 2 * self.P + 2)
            act(t_xc[:], xc_ps[:], AF.Silu, bias=self.cols[mi][:, 0:1])
            xc[mi] = t_xc

            z_ps = _tile(ps_mm, [128, CH], F32, "mm", 3)
            for kt in range(self.P_in):
                self.mm_in(nc, z_ps[:], self.wz[kt][:, ms],
                           self.in_planes[kt][:, c0 + 3: c0 + 3 + CH],
                           start=(kt == 0), stop=(kt == self.P_in - 1))
            t_sz = _tile(self.pools['sb'], [128, CH], BF16,
                         self.pfx + "sz", 2 * self.P + 2)
            act(t_sz[:], z_ps[:], AF.Silu, bias=self.cols[mi][:, 1:2])
            sz[mi] = t_sz
        s['xc'], s['sz'] = xc, sz
        self.st[k] = s

    # ---- B phase: wx/dt projections + dt chain --------------------------
    def front_b(self, k):
        nc, act = self.nc, self.act
        sb, ps_mm = self.pools['sb'], self.pools['mm']
        rw = self.r + 4
        s = self.st[k]
        xdbl_ps = _tile(ps_mm, [128, CH], F32, "mm", 3)
        for kt in range(self.P_in):
            _mmb(nc, xdbl_ps[:rw, :], self.wx[kt][:], s['xc'][kt][:],
                 start=(kt == 0), stop=(kt == self.P_in - 1))
        xdbl = _tile(sb, [rw, CH], BF16, self.pfx + "xdbl", 2)
        act(xdbl[:], xdbl_ps[:rw, :], AF.Identity)
        s['xdbl'] = xdbl

        dt_sb = []
        dA_sb = [[None] * self.P, [None] * self.P]
        for mi in range(self.P):
            ms = slice(mi * 128, (mi + 1) * 128)
            dt_ps = _tile(ps_mm, [128, CH], F32, "mm", 3)
            _mmb(nc, dt_ps[:], self.wdt[:, ms], xdbl[:self.r, :])
            t_e = _tile(sb, [128, CH], F32, "scr1", 3)
            act(t_e[:], dt_ps[:], AF.Exp, bias=self.cols[mi][:, 2:3])
            t_dt = _tile(sb, [128, CH], BF16, self.pfx + "dt", self.P + 2)
            act(t_dt[:], t_e[:], AF.Ln, bias=1.0)
            dt_sb.append(t_dt)
            t_dA0 = _tile(sb, [128, CH], F32, self.pfx + "dA0", self.P + 1)
            act(t_dA0[:], t_dt[:], AF.Exp, scale=self.cols[mi][:, 3:4])
            dA_sb[0][mi] = t_dA0
            t_dA1 = _tile(sb, [128, CH], F32, self.pfx + "dA1", self.P + 1)
            nc.gpsimd.tensor_mul(t_dA1[:], t_dA0[:], t_dA0[:])
            dA_sb[1][mi] = t_dA1
        s['dt'], s['dA'] = dt_sb, dA_sb
        rows = _tile(sb, [1, 4 * CH], BF16, "rows", 2)
        nc.scalar.dma_start(rows[:], xdbl[self.r:self.r + 4, :])
        rep_all = _tile(sb, [128, 4 * CH], BF16, "repall", 3)
        nc.gpsimd.partition_broadcast(rep_all[:], rows[:])
        s['rep'] = rep_all

    # ---- DVE: gate/scan chain -------------------------------------------
    def dve_chain(self, k):
        nc = self.nc
        sb = self.pools['sb']
        c0 = k * CH
        s = self.st[k]
        rep_all = s.pop('rep')
        rep = [rep_all[:, j * CH:(j + 1) * CH] for j in range(4)]
        hs = [[None] * self.P, [None] * self.P]
        for mi in range(self.P):
            t_u = _tile(sb, [128, CH], BF16, self.pfx + "scr2", 3)
            nc.vector.tensor_mul(t_u[:], s['dt'][mi][:], s['xc'][mi][:])
            for n in range(2):
                t_dbu = _tile(sb, [128, CH], BF16, self.pfx + f"dbu{n}", 2)
                nc.vector.tensor_mul(t_dbu[:], t_u[:], rep[n])
                t_hs = _tile(sb, [128, CH], BF16, self.pfx + f"hs{n}",
                             2 * self.P + 1)
                init = (0.0 if c0 == 0 else
                        self.hs_prev[n][mi][:, CH - 1:CH])
                nc.vector.tensor_tensor_scan(
                    t_hs[:], s['dA'][n][mi][:], t_dbu[:], init,
                    ALU.mult, ALU.add)
                hs[n][mi] = t_hs
                self.hs_prev[n][mi] = t_hs

        yg = []
        for mi in range(self.P):
            t_m0 = _tile(sb, [128, CH], BF16, self.pfx + "m0", 2)
            nc.vector.tensor_mul(t_m0[:], hs[0][mi][:], rep[2])
            t_y = _tile(sb, [128, CH], BF16, self.pfx + "y", 2)
            nc.vector.tensor_mul(t_y[:], hs[1][mi][:], rep[3])
            nc.vector.tensor_add(t_y[:], t_y[:], t_m0[:])
            t_dx = _tile(sb, [128, CH], BF16, self.pfx + "dx", 2)
            nc.vector.tensor_scalar(t_dx[:], s['xc'][mi][:],
                                    self.cols[mi][:, 5:6], None,
                                    ALU.mult, ALU.bypass)
            nc.vector.tensor_add(t_y[:], t_y[:], t_dx[:])
            t_yg = _tile(sb, [128, CH], BF16, self.pfx + "yg",
                         2 * self.P + 3)
            nc.vector.tensor_mul(t_yg[:], t_y[:], s['sz'][mi][:])
            yg.append(t_yg)
        s['yg'] = yg

    # ---- PE: out_proj into packed PSUM bank tiles -----------------------
    def out_proj(self, k):
        nc = self.nc
        s = self.st[k]
        views = []
        n_banks = (4 * self.dout) // 512       # s1: 1, s2: 2
        per_bank = 512 // self.dout            # s1: 4, s2: 2
        for b in range(n_banks):
            bank = _tile(self.ps_y, [SUB, 512], F32, "yb", 2)
            for h in range(per_bank):
                g = b * per_bank + h
                cs = slice(g * SUB, (g + 1) * SUB)
                dst = bank[:, h * self.dout:(h + 1) * self.dout]
                for mi in range(self.P):
                    nc.tensor.matmul(dst, s['yg'][mi][:, cs],
                                     self.wout[mi][:],
                                     start=(mi == 0), stop=(mi == self.P - 1))
                views.append(dst)
        s['yp'] = views

    # ---- B phase (retarded): yp -> SBUF + LN stats ----------------------
    def ln_acts(self, k):
        nc = self.nc
        sb = self.pools['sb']
        s = self.st[k]
        ssq4 = _tile(sb, [SUB, 4], F32, self.pfx + "ssq4", 2)
        ypsb = []
        for g in range(4):
            t_y = _tile(sb, [SUB, self.dout], BF16, self.pfx + "ypsb", 6)
            nc.scalar.activation(t_y[:], s['yp'][g], AF.Identity)
            scr = _tile(sb, [SUB, self.dout], BF16, self.pfx + "scr", 2)
            nc.scalar.activation(scr[:], t_y[:], AF.Square,
                                 accum_out=ssq4[:, g:g + 1])
            ypsb.append(t_y)
        s['ypsb'], s['ssq4'] = ypsb, ssq4

    # ---- Pool/DVE: rstd chain + normalize + emit ------------------------
    def ln_tail(self, k):
        nc = self.nc
        sb = self.pools['sb']
        s = self.st[k]
        c0 = k * CH
        dout = self.dout
        veps = _tile(sb, [SUB, 4], F32, self.pfx + "veps", 2)
        nc.gpsimd.tensor_scalar(veps[:], s['ssq4'][:], 1.0 / dout, LN_EPS,
                                ALU.mult, ALU.add)
        i1 = _tile(sb, [SUB, 4], I32, self.pfx + "i1", 2)
        nc.vector.tensor_scalar(i1[:], veps[:].bitcast(I32), 1, None,
                                ALU.arith_shift_right, ALU.bypass)
        nc.vector.tensor_scalar(i1[:], i1[:], 0xFFFFFFFF, None,
                                ALU.bitwise_xor, ALU.bypass)
        nc.vector.tensor_scalar(i1[:], i1[:], MAGIC + 1, None,
                                ALU.add, ALU.bypass)
        r0 = i1[:].bitcast(F32)
        vh = _tile(sb, [SUB, 4], F32, self.pfx + "vh", 2)
        nc.gpsimd.tensor_scalar(vh[:], veps[:], -0.5, None,
                                ALU.mult, ALU.bypass)
        for it in range(2):
            t_nr = _tile(sb, [SUB, 4], F32, self.pfx + f"nr{it}", 2)
            nc.gpsimd.tensor_mul(t_nr[:], r0, vh[:])
            nc.gpsimd.tensor_mul(t_nr[:], t_nr[:], r0)
            nc.gpsimd.tensor_scalar(t_nr[:], t_nr[:], 1.5, None,
                                    ALU.add, ALU.bypass)
            r1 = _tile(sb, [SUB, 4], F32, self.pfx + f"rst{it}", 2)
            nc.gpsimd.tensor_mul(r1[:], r0, t_nr[:])
            r0 = r1[:]
        tns = []
        for g in range(4):
            tn = _tile(sb, [SUB, dout], BF16, self.pfx + "tn", 8)
            nc.vector.tensor_scalar(tn[:], s['ypsb'][g][:],
                                    r0[:, g:g + 1], None,
                                    ALU.mult, ALU.bypass)
            tns.append(tn)
        s['tn'] = tns

    def ln_emit(self, k):
        """Transposes + feature-major copies, one iteration after ln_tail
        so the PE queue never waits on same-iteration DVE results."""
        s = self.st.pop(k)
        c0 = k * CH
        for g in range(4):
            self.emit(s['tn'][g], c0, g)
        self.flush(c0)


def build_program(L=4096):
    nc = bacc.Bacc()
    dp = nc.declare_dram_parameter
    x_d = dp("x", [128, L], F32R, isOutput=False)
    w1k_d = dp("w1k", [4, 128, 128], F32R, isOutput=False)
    w1z_d = dp("w1z", [128, 128], F32R, isOutput=False)
    wx1_d = dp("wx1", [128, 12], BF16, isOutput=False)
    wdt1_d = dp("wdt1", [8, 128], BF16, isOutput=False)
    wout1_d = dp("wout1", [128, 128], BF16, isOutput=False)
    cols1_d = dp("cols1", [128, 9], F32, isOutput=False)
    w2k_d = dp("w2k", [4, 256, 256], BF16, isOutput=False)
    w2z_d = dp("w2z", [256, 256], BF16, isOutput=False)
    wx2_d = dp("wx2", [256, 20], BF16, isOutput=False)
    wdt2_d = dp("wdt2", [16, 256], BF16, isOutput=False)
    wout2_d = dp("wout2", [256, 256], BF16, isOutput=False)
    cols2_d = dp("cols2", [256, 11], F32, isOutput=False)
    linw_d = dp("linw", [128, 256], BF16, isOutput=False)
    linb_d = dp("linb", [256, 1], F32, isOutput=False)
    iden_d = dp("idenb", [128, 128], BF16, isOutput=False)
    out_d = dp("out", [256, L], BF16, isOutput=True)

    dma = nc.sync.dma_start
    NCH = L // CH

    with tile.TileContext(nc) as tc, ExitStack() as ctx:
        consts = ctx.enter_context(tc.tile_pool(name="consts", bufs=1))
        planes = ctx.enter_context(tc.tile_pool(name="planes", bufs=1))
        sb = ctx.enter_context(tc.tile_pool(name="sb", bufs=2))
        ps_mm = ctx.enter_context(
            tc.tile_pool(name="psmm", bufs=3, space=bass.MemorySpace.PSUM))
        ps_y1 = ctx.enter_context(
            tc.tile_pool(name="psy1", bufs=2, space=bass.MemorySpace.PSUM))
        ps_y2 = ctx.enter_context(
            tc.tile_pool(name="psy2", bufs=2, space=bass.MemorySpace.PSUM))
        ps_tf = ctx.enter_context(
            tc.tile_pool(name="pstf", bufs=1, space=bass.MemorySpace.PSUM))
        pools = {'sb': sb, 'mm': ps_mm}

        xpad = planes.tile([128, L + 3], F32R, tag="xpad", name="xpad")
        nc.gpsimd.memset(xpad[:, 0:3].bitcast(F32), 0.0)
        for s0 in range(0, L, 2048):
            dma(xpad[:, 3 + s0: 3 + s0 + 2048], x_d[:, s0:s0 + 2048])

        _ld = [0]

        def load(dram_ap, shape, dtype=F32R):
            _ld[0] += 1
            t = consts.tile(shape, dtype, tag=f"w{_ld[0]}", name=f"w{_ld[0]}")
            dma(t[:], dram_ap)
            return t

        w1k_sb = [[load(w1k_d[k], [128, 128])] for k in range(4)]
        w1z_sb = [load(w1z_d[:], [128, 128])]
        wx1_sb = [load(wx1_d[:], [128, 12], BF16)]
        wdt1_sb = load(wdt1_d[:], [8, 128], BF16)
        wout1_sb = [load(wout1_d[:], [128, 128], BF16)]
        cols1_sb = [load(cols1_d[:], [128, 9], F32)]
        w2k_sb = [[load(w2k_d[k, kt * 128:(kt + 1) * 128], [128, 256], BF16)
                   for kt in range(2)] for k in range(4)]
        w2z_sb = [load(w2z_d[kt * 128:(kt + 1) * 128], [128, 256], BF16)
                  for kt in range(2)]
        wx2_sb = [load(wx2_d[kt * 128:(kt + 1) * 128], [128, 20], BF16)
                  for kt in range(2)]
        wdt2_sb = load(wdt2_d[:], [16, 256], BF16)
        wout2_sb = [load(wout2_d[kt * 128:(kt + 1) * 128], [128, 256], BF16)
                    for kt in range(2)]
        cols2_sb = [load(cols2_d[kt * 128:(kt + 1) * 128], [128, 11], F32)
                    for kt in range(2)]
        linw_sb = load(linw_d[:], [128, 256], BF16)
        linb_sb = [load(linb_d[kt * 128:(kt + 1) * 128], [128, 1], F32)
                   for kt in range(2)]

        bar_tile = consts.tile([1, 1], F32, tag="actbar", name="actbar")
        nc.gpsimd.memset(bar_tile[:], 0.0)
        act_chain = _ActChain(nc, bar_tile)
        ident = consts.tile([128, 128], BF16, tag="ident", name="ident")
        dma(ident[:], iden_d[:])

        t1n = planes.tile([128, L], BF16, tag="t1n", name="t1n")
        t2pad = [planes.tile([128, L + 3], BF16, tag=f"t2pad{mi}",
                             name=f"t2pad{mi}") for mi in range(2)]
        for mi in range(2):
            nc.gpsimd.memset(t2pad[mi][:, 0:3], 0.0)

        # ---- emits ----
        def emit1(tn, c0, g):
            tf = _tile(ps_tf, [128, 2 * SUB], BF16, "tf", 1)
            nc.tensor.transpose(tf[:, :SUB], tn[:], ident[:])
            nc.vector.tensor_scalar(
                t1n[:, c0 + g * SUB: c0 + (g + 1) * SUB], tf[:, :SUB],
                1.0, None, ALU.mult, ALU.bypass)

        of_buf = [None, None]

        def emit2(tn, c0, g):
            tf = _tile(ps_tf, [128, 2 * SUB], BF16, "tf", 1)
            for ct in range(2):
                if g == 0:
                    of_buf[ct] = _tile(sb, [128, CH], BF16, f"of{ct}", 2)
                nc.tensor.transpose(tf[:, ct * SUB:(ct + 1) * SUB],
                                    tn[:, ct * 128:(ct + 1) * 128],
                                    ident[:])
                nc.vector.tensor_scalar(
                    of_buf[ct][:, g * SUB:(g + 1) * SUB],
                    tf[:, ct * SUB:(ct + 1) * SUB],
                    cols2_sb[ct][:, 9:10], cols2_sb[ct][:, 10:11],
                    ALU.mult, ALU.add)

        def flush2(c0):
            for ct in range(2):
                dma(out_d[ct * 128:(ct + 1) * 128, c0:c0 + CH],
                    of_buf[ct][:])

        s1 = _Stage(nc, act_chain, pools, 1, 1, 8, 128, [xpad],
                    w1k_sb, w1z_sb, wx1_sb, wdt1_sb, wout1_sb, cols1_sb,
                    _mmr, ps_y1, emit1, lambda c0: None, pfx="a")
        s2 = _Stage(nc, act_chain, pools, 2, 2, 16, 256, t2pad,
                    w2k_sb, w2z_sb, wx2_sb, wdt2_sb, wout2_sb, cols2_sb,
                    _mmb, ps_y2, emit2, flush2, pfx="b")

        def linear_a(k):
            c0 = k * CH
            for mi in range(2):
                ms = slice(mi * 128, (mi + 1) * 128)
                lp = _tile(ps_mm, [128, CH], F32, "mm", 3)
                _mmb(nc, lp[:], linw_sb[:, ms], t1n[:, c0:c0 + CH])
                act_chain(t2pad[mi][:, 3 + c0: 3 + c0 + CH], lp[:],
                          AF.Silu, bias=linb_sb[mi][:, 0:1])

        # ---- the interleaved chunk pipeline ----
        # iter k: s1 front(k); s2 front(k-3); LN stats retarded by TWO
        # chunks (s1: k-2, s2: k-5) so every ACT-chained op is at least one
        # full iteration old when the in-order ACT queue reaches it.
        # out_proj runs at the PE stream front for the same reason.
        for k in range(NCH + 8):
            if 0 <= k - 3 < NCH:                  # emits first: PE stream
                s1.ln_emit(k - 3)                 # front never waits on DVE
            if 0 <= k - 7 < NCH:
                s2.ln_emit(k - 7)
            act_chain.new_group()                 # A phase (Silu table)
            if k < NCH:
                s1.front_a(k)
            if 0 <= k - 4 < NCH:
                linear_a(k - 4)
            if 0 <= k - 4 < NCH:
                s2.front_a(k - 4)
            act_chain.new_group()                 # B phase (Exp/Ln table)
            if k < NCH:
                s1.front_b(k)
            if 0 <= k - 4 < NCH:
                s2.front_b(k - 4)
            if 0 <= k - 2 < NCH:                  # retarded LN stats (free)
                s1.ln_acts(k - 2)
            if 0 <= k - 6 < NCH:
                s2.ln_acts(k - 6)
            if 0 <= k - 1 < NCH:                  # out_proj at PE tail
                s1.out_proj(k - 1)
            if 0 <= k - 5 < NCH:
                s2.out_proj(k - 5)
            if k < NCH:
                s1.dve_chain(k)
            if 0 <= k - 4 < NCH:
                s2.dve_chain(k - 4)
            if 0 <= k - 2 < NCH:
                s1.ln_tail(k - 2)
            if 0 <= k - 6 < NCH:
                s2.ln_tail(k - 6)

    nc.finalize()
    return nc


# ----------------------------------------------------------------------------
# entry point
# ----------------------------------------------------------------------------

_NC = {}


def kernel(**inputs):
    global last_exec_time_ns, last_results
    inputs = {k: np.asarray(v) for k, v in inputs.items()}
    weights = prep_weights(inputs)
    x = inputs['x'].astype(np.float32)          # [8, 128, 64, 64]
    b, c, h, w = x.shape
    L = h * w

    a1 = -np.exp(np.asarray(inputs['s1_alog'], np.float32))
    a2 = -np.exp(np.asarray(inputs['s2_alog'], np.float32))
    assert (np.allclose(a1[:, 1], 2 * a1[:, 0], rtol=1e-6) and
            np.allclose(a2[:, 1], 2 * a2[:, 0], rtol=1e-6)), \
        "kernel assumes A2 == 2*A1 (dA1 = dA0^2)"
    if L not in _NC:
        _NC[L] = build_program(L)

    in_maps = [dict(weights, x=np.ascontiguousarray(x[i].reshape(c, L)))
               for i in range(NCORES)]
    res = run_bass_kernel_spmd(
        _NC[L], in_maps, list(range(NCORES)),
        trace=bool(os.environ.get("KBENCH_TRACE")))
    last_exec_time_ns = res.exec_time_ns
    last_results = res
    out = np.stack([np.asarray(res.results[i]['out'], np.float32)
                    .reshape(256, h, w) for i in range(NCORES)])
    return out
